# revision 1
# baseline (speedup 1.0000x reference)
"""GQA kernel for 8 Trainium2 NeuronCores.

Problem: nn_GroupQueryAttention — B=2, S=2048, HIDDEN=2048, 32 heads,
8 kv-groups, head_dim 64.

Sharding: data parallel on batch (2) x tensor parallel on kv-groups (4
group-pairs). Core c owns batch c//4 and kv-groups {2*(c%4), 2*(c%4)+1}
(512 q-features, 128 kv-features). Each core computes a partial
out-projection (Wo columns of its features); host sums 4 partials per
batch.

Layout strategy (everything transposed on host so no on-chip transposes
are needed):
  - xT      (H, S)   : projections contract over hidden => hidden on
                       partitions for both operands.
  - qT/kT   (feat,S) : scores^T[k,q] = sum_d kT[d,k]*qT[d,q], computed
                       with k-positions on PSUM partitions so the mask
                       (per key) is a native per-partition ACT bias and
                       exp is fused: E = exp(scale*s + maskbias).
  - v       (S,feat) : PV matmul attnT[f,q] = sum_k v[k,f]*E[k,q] with an
                       extra ones-column in v producing the softmax
                       denominator as row 64 of the PSUM tile.
  - division by the denominator: reciprocal of the denom row, broadcast
    across partitions with a K=1 matmul against a ones column.
All matmuls run as float32r (fp32 bits, full PE rate at moving dim>=256).
"""

import numpy as np

B = 2
S = 2048
H = 2048
G = 8            # kv groups total
HPG = 4          # heads per group
D = 64           # head dim
NCORES = 8
QF = 512         # q features per core (2 groups * 4 heads * 64)
KF = 128         # kv features per core (2 groups * 64)
SCALE = 1.0 / np.sqrt(np.float32(D))
P = 128
SQA = 256        # seq chunk for projection phase (moving dim)
SQB = 512        # q tile for attention / out-proj phase
NKB = S // P     # 16 key blocks
NHT = H // P     # 16 hidden partition tiles
MASK_NEG = -1.0e5


def _build_bass():
    import concourse.tile as tile
    from concourse import bacc, mybir

    f32 = mybir.dt.float32
    f32r = mybir.dt.float32r
    Exp = mybir.ActivationFunctionType.Exp

    nc = bacc.Bacc("TRN2", target_bir_lowering=False, debug=False)

    xT = nc.dram_tensor("xT", [H, S], f32r, kind="ExternalInput").ap()
    wqT = nc.dram_tensor("wqT", [H, QF], f32r, kind="ExternalInput").ap()
    wkT = nc.dram_tensor("wkT", [H, KF], f32r, kind="ExternalInput").ap()
    wvT = nc.dram_tensor("wvT", [H, KF], f32r, kind="ExternalInput").ap()
    woT = nc.dram_tensor("woT", [QF, H], f32r, kind="ExternalInput").ap()
    mb = nc.dram_tensor("mb", [P, NKB], f32, kind="ExternalInput").ap()
    outT = nc.dram_tensor("outT", [H, S], f32, kind="ExternalOutput").ap()

    with tile.TileContext(nc) as tc:
        with (
            # f32r tiles are fp32 bits pre-rounded for the PE; reductions
            # still accumulate in fp32 PSUM.
            nc.allow_low_precision(reason="float32r rounding for PE matmuls"),
            tc.tile_pool(name="const", bufs=1) as const_pool,
            tc.tile_pool(name="wbig", bufs=1) as wbig_pool,
            tc.tile_pool(name="wkv", bufs=1) as wkv_pool,
            tc.tile_pool(name="xt", bufs=2) as xt_pool,
            tc.tile_pool(name="qt", bufs=1) as qt_pool,
            tc.tile_pool(name="kt", bufs=1) as kt_pool,
            tc.tile_pool(name="vs", bufs=1) as v_pool,
            tc.tile_pool(name="at", bufs=2) as at_pool,
            tc.tile_pool(name="e", bufs=6) as e_pool,
            tc.tile_pool(name="rc", bufs=2) as rc_pool,
            tc.tile_pool(name="rb", bufs=2) as rb_pool,
            tc.tile_pool(name="outs", bufs=3) as out_pool,
            tc.tile_pool(name="psa", bufs=3, space="PSUM") as psa_pool,
            tc.tile_pool(name="pso", bufs=2, space="PSUM") as pso_pool,
            tc.tile_pool(name="psb", bufs=2, space="PSUM") as psb_pool,
        ):
            # ---- constants ----
            mb_sb = const_pool.tile([P, NKB], f32, tag="mb")
            nc.sync.dma_start(out=mb_sb, in_=mb)
            ones32_sb = const_pool.tile([1, D], f32, tag="ones32")
            nc.vector.memset(ones32_sb, 1.0)
            ones_sb = const_pool.tile([1, D], f32r, tag="ones")
            nc.vector.tensor_copy(ones_sb, ones32_sb)

            # ---- weights ----
            wq_sb = wbig_pool.tile([P, NHT, QF], f32r, tag="wbig")
            nc.sync.dma_start(
                out=wq_sb, in_=wqT.rearrange("(t p) f -> p t f", p=P)
            )
            wk_sb = wkv_pool.tile([P, NHT, KF], f32r, tag="wk")
            wv_sb = wkv_pool.tile([P, NHT, KF], f32r, tag="wv")
            nc.sync.dma_start(
                out=wk_sb, in_=wkT.rearrange("(t p) f -> p t f", p=P)
            )
            nc.sync.dma_start(
                out=wv_sb, in_=wvT.rearrange("(t p) f -> p t f", p=P)
            )

            qt_sb = qt_pool.tile([P, QF // P, S], f32r, tag="qt")
            # kT stored twice: kta = [g0; g1] on partitions [0:64; 64:128],
            # ktb = [g1; g0] — so any (group, q-parity) pair can be read at
            # the base partition matmul requires (lhsT base == rhs base).
            kta_sb = kt_pool.tile([P, S], f32r, tag="kta")
            ktb_sb = kt_pool.tile([P, S], f32r, tag="ktb")
            # v layout: [g0 v (64) | ones | g1 v (64) | ones] per key block
            v_sb = v_pool.tile([P, NKB, 130], f32r, tag="v")
            onescol_sb = const_pool.tile([P, NKB], f32, tag="onescol")
            nc.vector.memset(onescol_sb, 1.0)
            nc.vector.tensor_copy(v_sb[:, :, 64], onescol_sb)
            nc.vector.tensor_copy(v_sb[:, :, 129], onescol_sb)

            # ---- phase A: projections (contract over hidden) ----
            nsq = S // SQA
            for sq in range(nsq):
                s0 = sq * SQA
                xt = xt_pool.tile([P, NHT, SQA], f32r, tag="xt")
                nc.sync.dma_start(
                    out=xt,
                    in_=xT.rearrange("(t p) s -> p t s", p=P)[:, :, s0:s0 + SQA],
                )
                # qT (4 feature ptiles)
                for mt in range(QF // P):
                    ps = psa_pool.tile([P, SQA], f32, tag="ps")
                    for ht in range(NHT):
                        nc.tensor.matmul(
                            ps,
                            lhsT=wq_sb[:, ht, mt * P:(mt + 1) * P],
                            rhs=xt[:, ht, :],
                            start=(ht == 0),
                            stop=(ht == NHT - 1),
                        )
                    nc.vector.tensor_copy(qt_sb[:, mt, s0:s0 + SQA], ps)
                # kT
                ps = psa_pool.tile([P, SQA], f32, tag="ps")
                for ht in range(NHT):
                    nc.tensor.matmul(
                        ps,
                        lhsT=wk_sb[:, ht, :],
                        rhs=xt[:, ht, :],
                        start=(ht == 0),
                        stop=(ht == NHT - 1),
                    )
                nc.vector.tensor_copy(kta_sb[:, s0:s0 + SQA], ps)
                nc.vector.tensor_copy(ktb_sb[0:64, s0:s0 + SQA], ps[64:128, :])
                nc.vector.tensor_copy(ktb_sb[64:128, s0:s0 + SQA], ps[0:64, :])
                # v (seq-major): out[s, vf]
                for st in range(SQA // P):
                    kb = (s0 + st * P) // P
                    psv = psa_pool.tile([P, KF], f32, tag="ps")
                    for ht in range(NHT):
                        nc.tensor.matmul(
                            psv,
                            lhsT=xt[:, ht, st * P:(st + 1) * P],
                            rhs=wv_sb[:, ht, :],
                            start=(ht == 0),
                            stop=(ht == NHT - 1),
                        )
                    nc.vector.tensor_copy(v_sb[:, kb, 0:64], psv[:, 0:64])
                    nc.vector.tensor_copy(v_sb[:, kb, 65:129], psv[:, 64:128])

            # out-proj weights (reuses the wq slot once phase A is done)
            wo_sb = wbig_pool.tile([P, QF // P, H], f32r, tag="wbig")
            nc.sync.dma_start(
                out=wo_sb, in_=woT.rearrange("(t p) f -> p t f", p=P)
            )

            # ---- phase B/C: attention + out-projection per q tile ----
            for qt in range(S // SQB):
                q0 = qt * SQB
                at = at_pool.tile([P, QF // P, SQB], f32r, tag="at")
                for h in range(2 * HPG):
                    g = h // HPG
                    mt, r0 = divmod(h, 2)
                    r0 *= D
                    par = r0 // D  # q-head parity: base partition 0 or 64
                    kt_src = kta_sb if g == par else ktb_sb
                    po = pso_pool.tile([65, SQB], f32, tag="po")
                    for kb in range(NKB):
                        ps = psa_pool.tile([P, SQB], f32, tag="ps")
                        nc.tensor.matmul(
                            ps,
                            lhsT=kt_src[r0:r0 + D, kb * P:(kb + 1) * P],
                            rhs=qt_sb[r0:r0 + D, mt, q0:q0 + SQB],
                            start=True,
                            stop=True,
                        )
                        e = e_pool.tile([P, SQB], f32r, tag="e")
                        nc.scalar.activation(
                            e, ps, Exp,
                            bias=mb_sb[:, kb:kb + 1], scale=float(SCALE),
                        )
                        nc.tensor.matmul(
                            po,
                            lhsT=v_sb[:, kb, g * 65:(g + 1) * 65],
                            rhs=e,
                            start=(kb == 0),
                            stop=(kb == NKB - 1),
                        )
                    # normalize: rows 0..63 are numerator^T, row 64 denominator
                    rc = rc_pool.tile([1, SQB], f32r, tag="rc")
                    nc.vector.reciprocal(rc, po[64:65, :])
                    pb = psb_pool.tile([D, SQB], f32, tag="pb")
                    nc.tensor.matmul(
                        pb, lhsT=ones_sb, rhs=rc, start=True, stop=True
                    )
                    rb = rb_pool.tile([D, SQB], f32, tag="rb")
                    nc.scalar.copy(rb, pb)
                    nc.vector.tensor_mul(at[r0:r0 + D, mt, :], po[0:64, :], rb)
                # out-projection for this q tile
                for mt in range(NHT):
                    ps = psa_pool.tile([P, SQB], f32, tag="ps")
                    for kb4 in range(QF // P):
                        nc.tensor.matmul(
                            ps,
                            lhsT=wo_sb[:, kb4, mt * P:(mt + 1) * P],
                            rhs=at[:, kb4, :],
                            start=(kb4 == 0),
                            stop=(kb4 == QF // P - 1),
                        )
                    ot = out_pool.tile([P, SQB], f32, tag="ot")
                    nc.vector.tensor_copy(ot, ps)
                    nc.sync.dma_start(
                        out=outT[mt * P:(mt + 1) * P, q0:q0 + SQB], in_=ot
                    )
    nc.compile()
    return nc


_NC_CACHE = None


def _get_nc():
    global _NC_CACHE
    if _NC_CACHE is None:
        _NC_CACHE = _build_bass()
    return _NC_CACHE


def _make_in_maps(inputs):
    x = np.asarray(inputs["x"], dtype=np.float32)
    mask = np.asarray(inputs["mask"])
    Wq = np.asarray(inputs["Wq"], dtype=np.float32)
    Wk = np.asarray(inputs["Wk"], dtype=np.float32)
    Wv = np.asarray(inputs["Wv"], dtype=np.float32)
    Wo = np.asarray(inputs["Wo"], dtype=np.float32)

    xTs = [np.ascontiguousarray(x[b].T) for b in range(B)]
    mbs = []
    for b in range(B):
        m = mask[b, 0, 0, 0, :]
        bias = np.where(m == 0, np.float32(MASK_NEG), np.float32(0.0))
        mbs.append(np.ascontiguousarray(bias.astype(np.float32).reshape(NKB, P).T))

    in_maps = []
    for c in range(NCORES):
        b, gp = divmod(c, 4)
        qs = slice(gp * QF, (gp + 1) * QF)
        ks = slice(gp * KF, (gp + 1) * KF)
        in_maps.append({
            "xT": xTs[b],
            "wqT": np.ascontiguousarray(Wq[qs, :].T),
            "wkT": np.ascontiguousarray(Wk[ks, :].T),
            "wvT": np.ascontiguousarray(Wv[ks, :].T),
            "woT": np.ascontiguousarray(Wo[:, qs].T),
            "mb": mbs[b],
        })
    return in_maps


def kernel(**inputs):
    from concourse.bass_utils import run_bass_kernel_spmd

    in_maps = _make_in_maps(inputs)
    nc = _get_nc()
    res = run_bass_kernel_spmd(nc, in_maps, core_ids=list(range(NCORES)))
    outs = [r["outT"] for r in res.results]
    out = np.empty((B, S, H), dtype=np.float32)
    for b in range(B):
        acc = outs[4 * b] + outs[4 * b + 1] + outs[4 * b + 2] + outs[4 * b + 3]
        out[b] = acc.T
    return out



# revision 3
# speedup vs baseline: 2.0690x; 2.0690x over previous
"""GQA kernel for 8 Trainium2 NeuronCores.

Problem: nn_GroupQueryAttention — B=2, S=2048, HIDDEN=2048, 32 heads,
8 kv-groups, head_dim 64.

Sharding: data parallel on batch (2) x tensor parallel on kv-groups (4
group-pairs). Core c owns batch c//4 and kv-groups {2*(c%4), 2*(c%4)+1}
(512 q-features, 128 kv-features). Each core computes a partial
out-projection (Wo columns of its features); host sums 4 partials per
batch.

Key optimizations over the fp32r baseline:
  - all matmul operands in bf16 (fp32r ran under a HW power throttle at
    ~1.2GHz; bf16 streams 1 col/cycle at full clock). PSUM accumulation
    stays fp32.
  - key compaction: the mask is per-key (broadcast over queries+heads),
    so masked keys contribute nothing anywhere. Host gathers the
    unmasked key positions, pads to a multiple of 128, and the kernel
    only projects/attends over the kept keys (~half of 2048 for the
    random mask). Padded key rows get an exp bias of -30000 so E=0.
  - reciprocal_approx_fast for the softmax denominators.
  - Wo loaded at start so its DMA overlaps phase A.

Layout strategy (everything transposed on host so no on-chip transposes
are needed):
  - xT      (H, S)    : q projection contracts over hidden.
  - xkT     (H, KPAD) : gathered keys' x for k/v projections.
  - qT/kT   (feat,S)  : scores^T[k,q] = sum_d kT[d,k]*qT[d,q], with
                        k-positions on PSUM partitions so the pad mask is
                        a per-partition ACT bias and exp is fused.
  - v       (KPAD,f)  : PV matmul attnT[f,q] = sum_k v[k,f]*E[k,q] with an
                        extra ones-column producing the softmax
                        denominator as row 64 of the PSUM tile.
  - division by the denominator: approx reciprocal of the denom row,
    broadcast across partitions with a K=1 matmul against a ones column.
"""

import numpy as np

B = 2
S = 2048
H = 2048
G = 8            # kv groups total
HPG = 4          # heads per group
D = 64           # head dim
NCORES = 8
QF = 512         # q features per core (2 groups * 4 heads * 64)
KF = 128         # kv features per core (2 groups * 64)
SCALE = 1.0 / np.sqrt(np.float32(D))
P = 128
SQA = 512        # seq chunk for projection phase (moving dim)
SQB = 512        # q tile for attention / out-proj phase
NHT = H // P     # 16 hidden partition tiles
MASK_NEG = -30000.0


def _build_bass(KB):
    """Build the per-core program for KPAD = KB*128 kept+padded keys."""
    import concourse.tile as tile
    from concourse import bacc, mybir

    f32 = mybir.dt.float32
    bf16 = mybir.dt.bfloat16
    Exp = mybir.ActivationFunctionType.Exp
    KPAD = KB * P

    nc = bacc.Bacc("TRN2", target_bir_lowering=False, debug=False)

    xT = nc.dram_tensor("xT", [H, S], bf16, kind="ExternalInput").ap()
    xkT = nc.dram_tensor("xkT", [H, KPAD], bf16, kind="ExternalInput").ap()
    wqT = nc.dram_tensor("wqT", [H, QF], bf16, kind="ExternalInput").ap()
    wkT = nc.dram_tensor("wkT", [H, KF], bf16, kind="ExternalInput").ap()
    wvT = nc.dram_tensor("wvT", [H, KF], bf16, kind="ExternalInput").ap()
    woT = nc.dram_tensor("woT", [QF, H], bf16, kind="ExternalInput").ap()
    mb = nc.dram_tensor("mb", [P, KB], f32, kind="ExternalInput").ap()
    outT = nc.dram_tensor("outT", [H, S], f32, kind="ExternalOutput").ap()

    with tile.TileContext(nc) as tc:
        with (
            nc.allow_low_precision(reason="bf16 matmuls, fp32 PSUM accum"),
            tc.tile_pool(name="const", bufs=1) as const_pool,
            tc.tile_pool(name="wq", bufs=1) as wq_pool,
            tc.tile_pool(name="wo", bufs=1) as wo_pool,
            tc.tile_pool(name="wkv", bufs=1) as wkv_pool,
            tc.tile_pool(name="xt", bufs=2) as xt_pool,
            tc.tile_pool(name="qt", bufs=1) as qt_pool,
            tc.tile_pool(name="kt", bufs=1) as kt_pool,
            tc.tile_pool(name="vs", bufs=1) as v_pool,
            tc.tile_pool(name="at", bufs=2) as at_pool,
            tc.tile_pool(name="e", bufs=6) as e_pool,
            tc.tile_pool(name="rc", bufs=2) as rc_pool,
            tc.tile_pool(name="rb", bufs=2) as rb_pool,
            tc.tile_pool(name="outs", bufs=3) as out_pool,
            tc.tile_pool(name="psa", bufs=3, space="PSUM") as psa_pool,
            tc.tile_pool(name="pso", bufs=2, space="PSUM") as pso_pool,
            tc.tile_pool(name="psb", bufs=2, space="PSUM") as psb_pool,
        ):
            # ---- constants ----
            mb_sb = const_pool.tile([P, KB], f32, tag="mb")
            nc.sync.dma_start(out=mb_sb, in_=mb)
            ones_sb = const_pool.tile([1, D], bf16, tag="ones")
            nc.vector.memset(ones_sb, 1.0)

            # ---- weights (all loaded up-front; DMA overlaps phase A) ----
            wq_sb = wq_pool.tile([P, NHT, QF], bf16, tag="wq")
            nc.sync.dma_start(
                out=wq_sb, in_=wqT.rearrange("(t p) f -> p t f", p=P)
            )
            wk_sb = wkv_pool.tile([P, NHT, KF], bf16, tag="wk")
            wv_sb = wkv_pool.tile([P, NHT, KF], bf16, tag="wv")
            nc.sync.dma_start(
                out=wk_sb, in_=wkT.rearrange("(t p) f -> p t f", p=P)
            )
            nc.sync.dma_start(
                out=wv_sb, in_=wvT.rearrange("(t p) f -> p t f", p=P)
            )
            wo_sb = wo_pool.tile([P, QF // P, H], bf16, tag="wo")
            nc.sync.dma_start(
                out=wo_sb, in_=woT.rearrange("(t p) f -> p t f", p=P)
            )

            qt_sb = qt_pool.tile([P, QF // P, S], bf16, tag="qt")
            # kT stored twice: kta = [g0; g1] on partitions [0:64; 64:128],
            # ktb = [g1; g0] — so any (group, q-parity) pair can be read at
            # the base partition (matmul requires lhsT base == rhs base).
            kta_sb = kt_pool.tile([P, KPAD], bf16, tag="kta")
            ktb_sb = kt_pool.tile([P, KPAD], bf16, tag="ktb")
            # v layout: [g0 v (64) | ones | g1 v (64) | ones] per key block
            v_sb = v_pool.tile([P, KB, 130], bf16, tag="v")
            onescol_sb = const_pool.tile([P, KB], bf16, tag="onescol")
            nc.vector.memset(onescol_sb, 1.0)
            nc.vector.tensor_copy(v_sb[:, :, 64], onescol_sb)
            nc.vector.tensor_copy(v_sb[:, :, 129], onescol_sb)

            # ---- phase A1: q projection (contract over hidden) ----
            for sq in range(S // SQA):
                s0 = sq * SQA
                xt = xt_pool.tile([P, NHT, SQA], bf16, tag="xt")
                nc.sync.dma_start(
                    out=xt,
                    in_=xT.rearrange("(t p) s -> p t s", p=P)[:, :, s0:s0 + SQA],
                )
                for mt in range(QF // P):
                    ps = psa_pool.tile([P, SQA], f32, tag="ps")
                    for ht in range(NHT):
                        nc.tensor.matmul(
                            ps,
                            lhsT=wq_sb[:, ht, mt * P:(mt + 1) * P],
                            rhs=xt[:, ht, :],
                            start=(ht == 0),
                            stop=(ht == NHT - 1),
                        )
                    nc.scalar.copy(qt_sb[:, mt, s0:s0 + SQA], ps)

            # ---- phase A2: k/v projections over kept keys ----
            for k0 in range(0, KPAD, SQA):
                kw = min(SQA, KPAD - k0)
                xk = xt_pool.tile([P, NHT, SQA], bf16, tag="xt")
                nc.sync.dma_start(
                    out=xk[:, :, 0:kw],
                    in_=xkT.rearrange("(t p) s -> p t s", p=P)[:, :, k0:k0 + kw],
                )
                ps = psa_pool.tile([P, SQA], f32, tag="ps")
                for ht in range(NHT):
                    nc.tensor.matmul(
                        ps[:, 0:kw],
                        lhsT=wk_sb[:, ht, :],
                        rhs=xk[:, ht, 0:kw],
                        start=(ht == 0),
                        stop=(ht == NHT - 1),
                    )
                nc.scalar.copy(kta_sb[:, k0:k0 + kw], ps[:, 0:kw])
                nc.vector.tensor_copy(ktb_sb[0:64, k0:k0 + kw], ps[64:128, 0:kw])
                nc.vector.tensor_copy(ktb_sb[64:128, k0:k0 + kw], ps[0:64, 0:kw])
                # v (key-major): out[k, vf]
                for st in range(kw // P):
                    kb = (k0 + st * P) // P
                    psv = psa_pool.tile([P, KF], f32, tag="ps")
                    for ht in range(NHT):
                        nc.tensor.matmul(
                            psv,
                            lhsT=xk[:, ht, st * P:(st + 1) * P],
                            rhs=wv_sb[:, ht, :],
                            start=(ht == 0),
                            stop=(ht == NHT - 1),
                        )
                    nc.scalar.copy(v_sb[:, kb, 0:64], psv[:, 0:64])
                    nc.scalar.copy(v_sb[:, kb, 65:129], psv[:, 64:128])

            # ---- phase B/C: attention + out-projection per q tile ----
            for qt in range(S // SQB):
                q0 = qt * SQB
                at = at_pool.tile([P, QF // P, SQB], bf16, tag="at")
                for h in range(2 * HPG):
                    g = h // HPG
                    mt, r0 = divmod(h, 2)
                    r0 *= D
                    par = r0 // D  # q-head parity: base partition 0 or 64
                    kt_src = kta_sb if g == par else ktb_sb
                    po = pso_pool.tile([65, SQB], f32, tag="po")
                    for kb in range(KB):
                        ps = psa_pool.tile([P, SQB], f32, tag="ps")
                        nc.tensor.matmul(
                            ps,
                            lhsT=kt_src[r0:r0 + D, kb * P:(kb + 1) * P],
                            rhs=qt_sb[r0:r0 + D, mt, q0:q0 + SQB],
                            start=True,
                            stop=True,
                        )
                        e = e_pool.tile([P, SQB], bf16, tag="e")
                        nc.scalar.activation(
                            e, ps, Exp,
                            bias=mb_sb[:, kb:kb + 1], scale=float(SCALE),
                        )
                        nc.tensor.matmul(
                            po,
                            lhsT=v_sb[:, kb, g * 65:(g + 1) * 65],
                            rhs=e,
                            start=(kb == 0),
                            stop=(kb == KB - 1),
                        )
                    # normalize: rows 0..63 are numerator^T, row 64 denominator
                    # (copy denom to partition 0 first: custom-DVE recip
                    # mis-reads nonzero partition bases)
                    dn = rc_pool.tile([1, SQB], f32, tag="dn")
                    nc.vector.tensor_copy(dn, po[64:65, :])
                    rc = rc_pool.tile([1, SQB], f32, tag="rc")
                    nc.vector.reciprocal_approx_fast(rc, dn)
                    rcb = rc_pool.tile([1, SQB], bf16, tag="rcb")
                    nc.vector.tensor_copy(rcb, rc)
                    pb = psb_pool.tile([D, SQB], f32, tag="pb")
                    nc.tensor.matmul(
                        pb, lhsT=ones_sb, rhs=rcb, start=True, stop=True
                    )
                    rb = rb_pool.tile([D, SQB], f32, tag="rb")
                    nc.scalar.copy(rb, pb)
                    nc.vector.tensor_mul(at[r0:r0 + D, mt, :], po[0:64, :], rb)
                # out-projection for this q tile
                for mt in range(NHT):
                    ps = psa_pool.tile([P, SQB], f32, tag="ps")
                    for kb4 in range(QF // P):
                        nc.tensor.matmul(
                            ps,
                            lhsT=wo_sb[:, kb4, mt * P:(mt + 1) * P],
                            rhs=at[:, kb4, :],
                            start=(kb4 == 0),
                            stop=(kb4 == QF // P - 1),
                        )
                    ot = out_pool.tile([P, SQB], f32, tag="ot")
                    nc.vector.tensor_copy(ot, ps)
                    nc.sync.dma_start(
                        out=outT[mt * P:(mt + 1) * P, q0:q0 + SQB], in_=ot
                    )
    nc.compile()
    return nc


_NC_CACHE = {}


def _get_nc(KB):
    if KB not in _NC_CACHE:
        _NC_CACHE[KB] = _build_bass(KB)
    return _NC_CACHE[KB]


def _make_in_maps(inputs):
    import ml_dtypes

    bf = ml_dtypes.bfloat16
    x = np.asarray(inputs["x"], dtype=np.float32)
    mask = np.asarray(inputs["mask"])
    Wq = np.asarray(inputs["Wq"], dtype=np.float32)
    Wk = np.asarray(inputs["Wk"], dtype=np.float32)
    Wv = np.asarray(inputs["Wv"], dtype=np.float32)
    Wo = np.asarray(inputs["Wo"], dtype=np.float32)

    # gather kept (unmasked) key positions per batch; pad to common KPAD
    idxs = [np.nonzero(mask[b, 0, 0, 0, :] != 0)[0] for b in range(B)]
    kept_max = max(1, max(len(i) for i in idxs))
    KB = (kept_max + P - 1) // P
    KPAD = KB * P

    xTs, xkTs, mbs = [], [], []
    for b in range(B):
        xb = x[b].astype(bf)
        xTs.append(np.ascontiguousarray(xb.T))
        xk = np.zeros((KPAD, H), dtype=bf)
        xk[: len(idxs[b])] = xb[idxs[b]]
        xkTs.append(np.ascontiguousarray(xk.T))
        bias = np.full(KPAD, np.float32(MASK_NEG), dtype=np.float32)
        bias[: len(idxs[b])] = 0.0
        mbs.append(np.ascontiguousarray(bias.reshape(KB, P).T))

    in_maps = []
    for c in range(NCORES):
        b, gp = divmod(c, 4)
        qs = slice(gp * QF, (gp + 1) * QF)
        ks = slice(gp * KF, (gp + 1) * KF)
        in_maps.append({
            "xT": xTs[b],
            "xkT": xkTs[b],
            "wqT": np.ascontiguousarray(Wq[qs, :].T.astype(bf)),
            "wkT": np.ascontiguousarray(Wk[ks, :].T.astype(bf)),
            "wvT": np.ascontiguousarray(Wv[ks, :].T.astype(bf)),
            "woT": np.ascontiguousarray(Wo[:, qs].T.astype(bf)),
            "mb": mbs[b],
        })
    return in_maps, KB


def kernel(**inputs):
    from concourse.bass_utils import run_bass_kernel_spmd

    in_maps, KB = _make_in_maps(inputs)
    nc = _get_nc(KB)
    res = run_bass_kernel_spmd(nc, in_maps, core_ids=list(range(NCORES)))
    outs = [r["outT"] for r in res.results]
    out = np.empty((B, S, H), dtype=np.float32)
    for b in range(B):
        acc = outs[4 * b] + outs[4 * b + 1] + outs[4 * b + 2] + outs[4 * b + 3]
        out[b] = acc.T
    return out


# revision 11
# speedup vs baseline: 2.1835x; 1.0553x over previous
"""GQA kernel for 8 Trainium2 NeuronCores.

Problem: nn_GroupQueryAttention — B=2, S=2048, HIDDEN=2048, 32 heads,
8 kv-groups, head_dim 64.

Sharding: data parallel on batch (2) x tensor parallel on kv-groups (4
group-pairs). Core c owns batch c//4 and kv-groups {2*(c%4), 2*(c%4)+1}
(512 q-features, 128 kv-features). Each core computes a partial
out-projection (Wo columns of its features); host sums 4 partials per
batch.

Key optimizations over the fp32r baseline:
  - all matmul operands in bf16 (fp32r ran under a HW power throttle at
    ~1.2GHz; bf16 streams 1 col/cycle at full clock). PSUM accumulation
    stays fp32.
  - key compaction: the mask is per-key (broadcast over queries+heads),
    so masked keys contribute nothing anywhere. Host gathers the
    unmasked key positions, pads to a multiple of 128, and the kernel
    only projects/attends over the kept keys (~half of 2048 for the
    random mask). Padded key rows get an exp bias of -30000 so E=0.
  - reciprocal_approx_fast for the softmax denominators.
  - Wo loaded at start so its DMA overlaps phase A.

Layout strategy (everything transposed on host so no on-chip transposes
are needed):
  - xT      (H, S)    : q projection contracts over hidden.
  - xkT     (H, KPAD) : gathered keys' x for k/v projections.
  - qT/kT   (feat,S)  : scores^T[k,q] = sum_d kT[d,k]*qT[d,q], with
                        k-positions on PSUM partitions so the pad mask is
                        a per-partition ACT bias and exp is fused.
  - v       (KPAD,f)  : PV matmul attnT[f,q] = sum_k v[k,f]*E[k,q] with an
                        extra ones-column producing the softmax
                        denominator as row 64 of the PSUM tile.
  - division by the denominator: approx reciprocal of the denom row,
    broadcast across partitions with a K=1 matmul against a ones column.
"""

import numpy as np

B = 2
S = 2048
H = 2048
G = 8            # kv groups total
HPG = 4          # heads per group
D = 64           # head dim
NCORES = 8
QF = 512         # q features per core (2 groups * 4 heads * 64)
KF = 128         # kv features per core (2 groups * 64)
SCALE = 1.0 / np.sqrt(np.float32(D))
P = 128
SQA = 512        # seq chunk for projection phase (moving dim)
SQB = 512        # q tile for attention / out-proj phase
NHT = H // P     # 16 hidden partition tiles
MASK_NEG = -30000.0


def _build_bass(KB):
    """Build the per-core program for KPAD = KB*128 kept+padded keys."""
    import concourse.tile as tile
    from concourse import bacc, mybir

    f32 = mybir.dt.float32
    bf16 = mybir.dt.bfloat16
    Exp = mybir.ActivationFunctionType.Exp
    KPAD = KB * P

    nc = bacc.Bacc("TRN2", target_bir_lowering=False, debug=False)

    xT = nc.dram_tensor("xT", [H, S], bf16, kind="ExternalInput").ap()
    xkT = nc.dram_tensor("xkT", [H, KPAD], bf16, kind="ExternalInput").ap()
    wqT = nc.dram_tensor("wqT", [H, QF], bf16, kind="ExternalInput").ap()
    wkT = nc.dram_tensor("wkT", [H, KF], bf16, kind="ExternalInput").ap()
    wvT = nc.dram_tensor("wvT", [H, KF], bf16, kind="ExternalInput").ap()
    woT = nc.dram_tensor("woT", [QF, H], bf16, kind="ExternalInput").ap()
    mb = nc.dram_tensor("mb", [P, KB], f32, kind="ExternalInput").ap()
    outT = nc.dram_tensor("outT", [H, S], bf16, kind="ExternalOutput").ap()

    from contextlib import ExitStack

    with tile.TileContext(nc) as tc, ExitStack() as es:
        ec = es.enter_context
        ec(nc.allow_low_precision(reason="bf16 matmuls, fp32 PSUM accum"))
        const_pool = ec(tc.tile_pool(name="const", bufs=1))
        wq_pool = ec(tc.tile_pool(name="wq", bufs=1))
        wo_pool = ec(tc.tile_pool(name="wo", bufs=1))
        wkv_pool = ec(tc.tile_pool(name="wkv", bufs=1))
        xt_pool = ec(tc.tile_pool(name="xt", bufs=2))
        qt_pool = ec(tc.tile_pool(name="qt", bufs=1))
        kt_pool = ec(tc.tile_pool(name="kt", bufs=1))
        v_pool = ec(tc.tile_pool(name="vs", bufs=1))
        at_pool = ec(tc.tile_pool(name="at", bufs=2))
        e_pool = ec(tc.tile_pool(name="e", bufs=6))
        rc_pool = ec(tc.tile_pool(name="rc", bufs=2))
        rb_pool = ec(tc.tile_pool(name="rb", bufs=2))
        out_pool = ec(tc.tile_pool(name="outs", bufs=3))
        psa_pool = ec(tc.tile_pool(name="psa", bufs=2, space="PSUM"))
        pso_pool = ec(tc.tile_pool(name="pso", bufs=1, space="PSUM"))
        psb_pool = ec(tc.tile_pool(name="psb", bufs=1, space="PSUM"))
        if True:
            # ---- constants ----
            mb_sb = const_pool.tile([P, KB], f32, tag="mb")
            nc.sync.dma_start(out=mb_sb, in_=mb)
            ones_sb = const_pool.tile([1, D], bf16, tag="ones")
            nc.vector.memset(ones_sb, 1.0)

            # ---- weights (all loaded up-front; DMA overlaps phase A) ----
            wq_sb = wq_pool.tile([P, NHT, QF], bf16, tag="wq")
            nc.sync.dma_start(
                out=wq_sb, in_=wqT.rearrange("(t p) f -> p t f", p=P)
            )
            wk_sb = wkv_pool.tile([P, NHT, KF], bf16, tag="wk")
            wv_sb = wkv_pool.tile([P, NHT, KF], bf16, tag="wv")
            nc.sync.dma_start(
                out=wk_sb, in_=wkT.rearrange("(t p) f -> p t f", p=P)
            )
            nc.sync.dma_start(
                out=wv_sb, in_=wvT.rearrange("(t p) f -> p t f", p=P)
            )
            wo_sb = wo_pool.tile([P, QF // P, H], bf16, tag="wo")
            nc.sync.dma_start(
                out=wo_sb, in_=woT.rearrange("(t p) f -> p t f", p=P)
            )

            # qT stored with col = (qtile, mt, q) so a head-pair's q tile is
            # one contiguous 2*SQB range (matmul moving AP must be 1D).
            qt_sb = qt_pool.tile([P, S * QF // P], bf16, tag="qt")
            # kT stored twice: kta = [g0; g1] on partitions [0:64; 64:128],
            # ktb = [g1; g0] — so any (group, q-parity) pair can be read at
            # the base partition (matmul requires lhsT base == rhs base).
            kta_sb = kt_pool.tile([P, KPAD], bf16, tag="kta")
            ktb_sb = kt_pool.tile([P, KPAD], bf16, tag="ktb")
            # v layout: [g0 v (64) | ones | g1 v (64) | ones] per key block
            v_sb = v_pool.tile([P, KB, 130], bf16, tag="v")
            onescol_sb = const_pool.tile([P, KB], bf16, tag="onescol")
            nc.vector.memset(onescol_sb, 1.0)
            nc.vector.tensor_copy(v_sb[:, :, 64], onescol_sb)
            nc.vector.tensor_copy(v_sb[:, :, 129], onescol_sb)

            # ---- phase A1: q projection (contract over hidden) ----
            for sq in range(S // SQA):
                s0 = sq * SQA
                xt = xt_pool.tile([P, NHT, SQA], bf16, tag="xt")
                nc.sync.dma_start(
                    out=xt,
                    in_=xT.rearrange("(t p) s -> p t s", p=P)[:, :, s0:s0 + SQA],
                )
                for mt in range(QF // P):
                    ps = psa_pool.tile([P, SQA], f32, tag="ps")
                    for ht in range(NHT):
                        nc.tensor.matmul(
                            ps,
                            lhsT=wq_sb[:, ht, mt * P:(mt + 1) * P],
                            rhs=xt[:, ht, :],
                            start=(ht == 0),
                            stop=(ht == NHT - 1),
                        )
                    c0 = (sq * (QF // P) + mt) * SQA
                    nc.scalar.copy(qt_sb[:, c0:c0 + SQA], ps)

            # ---- phase A2: k/v projections over kept keys ----
            for k0 in range(0, KPAD, SQA):
                kw = min(SQA, KPAD - k0)
                xk = xt_pool.tile([P, NHT, SQA], bf16, tag="xt")
                nc.sync.dma_start(
                    out=xk[:, :, 0:kw],
                    in_=xkT.rearrange("(t p) s -> p t s", p=P)[:, :, k0:k0 + kw],
                )
                ps = psa_pool.tile([P, SQA], f32, tag="ps")
                for ht in range(NHT):
                    nc.tensor.matmul(
                        ps[:, 0:kw],
                        lhsT=wk_sb[:, ht, :],
                        rhs=xk[:, ht, 0:kw],
                        start=(ht == 0),
                        stop=(ht == NHT - 1),
                    )
                nc.scalar.copy(kta_sb[:, k0:k0 + kw], ps[:, 0:kw])
                nc.vector.tensor_copy(ktb_sb[0:64, k0:k0 + kw], ps[64:128, 0:kw])
                nc.vector.tensor_copy(ktb_sb[64:128, k0:k0 + kw], ps[0:64, 0:kw])
                # v (key-major): out[k, vf]
                for st in range(kw // P):
                    kb = (k0 + st * P) // P
                    psv = psa_pool.tile([P, KF], f32, tag="ps")
                    for ht in range(NHT):
                        nc.tensor.matmul(
                            psv,
                            lhsT=xk[:, ht, st * P:(st + 1) * P],
                            rhs=wv_sb[:, ht, :],
                            start=(ht == 0),
                            stop=(ht == NHT - 1),
                        )
                    nc.scalar.copy(v_sb[:, kb, 0:64], psv[:, 0:64])
                    nc.scalar.copy(v_sb[:, kb, 65:129], psv[:, 64:128])

            # ---- phase B/C: attention + out-projection per q tile ----
            # Heads sharing a (group, q-parity) merge into one 2*SQB-wide
            # stream: pair hp covers heads at mt0=2*(hp//2)+{0,1}, partition
            # base r0=64*(hp%2), group g=hp//2.
            for qt in range(S // SQB):
                q0 = qt * SQB
                at = at_pool.tile([P, QF // P, SQB], bf16, tag="at")
                for hp in range(4):
                    g = hp // 2
                    par = hp % 2
                    r0 = par * D
                    mt0 = 2 * g
                    kt_src = kta_sb if g == par else ktb_sb
                    po = pso_pool.tile([65, 2 * SQB], f32, tag="po")
                    for kb in range(KB):
                        # matmul PSUM writes are bank-scoped (<=512 f32
                        # cols): two matmuls fill the wide tile's halves.
                        ps = psa_pool.tile([P, 2 * SQB], f32, tag="ps")
                        for j in range(2):
                            nc.tensor.matmul(
                                ps[:, j * SQB:(j + 1) * SQB],
                                lhsT=kt_src[r0:r0 + D, kb * P:(kb + 1) * P],
                                rhs=qt_sb[r0:r0 + D,
                                          (qt * 4 + mt0 + j) * SQB:
                                          (qt * 4 + mt0 + j + 1) * SQB],
                                start=True,
                                stop=True,
                            )
                        e = e_pool.tile([P, 2 * SQB], bf16, tag="e")
                        nc.scalar.activation(
                            e, ps, Exp,
                            bias=mb_sb[:, kb:kb + 1], scale=float(SCALE),
                        )
                        for j in range(2):
                            nc.tensor.matmul(
                                po[:, j * SQB:(j + 1) * SQB],
                                lhsT=v_sb[:, kb, g * 65:(g + 1) * 65],
                                rhs=e[:, j * SQB:(j + 1) * SQB],
                                start=(kb == 0),
                                stop=(kb == KB - 1),
                            )
                    # normalize: rows 0..63 are numerator^T, row 64 denominator
                    # (copy denom to partition 0 first: custom-DVE recip
                    # mis-reads nonzero partition bases)
                    dn = rc_pool.tile([1, 2 * SQB], f32, tag="dn")
                    nc.vector.tensor_copy(dn, po[64:65, :])
                    rc = rc_pool.tile([1, 2 * SQB], f32, tag="rc")
                    nc.vector.reciprocal_approx_fast(rc, dn)
                    rcb = rc_pool.tile([1, 2 * SQB], bf16, tag="rcb")
                    nc.vector.tensor_copy(rcb, rc)
                    pb = psb_pool.tile([D, 2 * SQB], f32, tag="pb")
                    for j in range(2):
                        nc.tensor.matmul(
                            pb[:, j * SQB:(j + 1) * SQB],
                            lhsT=ones_sb,
                            rhs=rcb[:, j * SQB:(j + 1) * SQB],
                            start=True,
                            stop=True,
                        )
                    rb = rb_pool.tile([D, 2 * SQB], f32, tag="rb")
                    nc.scalar.copy(rb, pb)
                    for j in range(2):
                        nc.vector.tensor_mul(
                            at[r0:r0 + D, mt0 + j, :],
                            po[0:64, j * SQB:(j + 1) * SQB],
                            rb[:, j * SQB:(j + 1) * SQB],
                        )
                # out-projection for this q tile
                for mt in range(NHT):
                    ps = psa_pool.tile([P, SQB], f32, tag="ps")
                    for kb4 in range(QF // P):
                        nc.tensor.matmul(
                            ps,
                            lhsT=wo_sb[:, kb4, mt * P:(mt + 1) * P],
                            rhs=at[:, kb4, :],
                            start=(kb4 == 0),
                            stop=(kb4 == QF // P - 1),
                        )
                    ot = out_pool.tile([P, SQB], bf16, tag="ot")
                    nc.vector.tensor_copy(ot, ps)
                    nc.sync.dma_start(
                        out=outT[mt * P:(mt + 1) * P, q0:q0 + SQB], in_=ot
                    )
    nc.compile()
    return nc


_NC_CACHE = {}


def _get_nc(KB):
    if KB not in _NC_CACHE:
        _NC_CACHE[KB] = _build_bass(KB)
    return _NC_CACHE[KB]


def _make_in_maps(inputs):
    import ml_dtypes

    bf = ml_dtypes.bfloat16
    x = np.asarray(inputs["x"], dtype=np.float32)
    mask = np.asarray(inputs["mask"])
    Wq = np.asarray(inputs["Wq"], dtype=np.float32)
    Wk = np.asarray(inputs["Wk"], dtype=np.float32)
    Wv = np.asarray(inputs["Wv"], dtype=np.float32)
    Wo = np.asarray(inputs["Wo"], dtype=np.float32)

    # gather kept (unmasked) key positions per batch; pad to common KPAD
    idxs = [np.nonzero(mask[b, 0, 0, 0, :] != 0)[0] for b in range(B)]
    kept_max = max(1, max(len(i) for i in idxs))
    KB = (kept_max + P - 1) // P
    KPAD = KB * P

    xTs, xkTs, mbs = [], [], []
    for b in range(B):
        xb = x[b].astype(bf)
        xTs.append(np.ascontiguousarray(xb.T))
        xk = np.zeros((KPAD, H), dtype=bf)
        xk[: len(idxs[b])] = xb[idxs[b]]
        xkTs.append(np.ascontiguousarray(xk.T))
        bias = np.full(KPAD, np.float32(MASK_NEG), dtype=np.float32)
        bias[: len(idxs[b])] = 0.0
        mbs.append(np.ascontiguousarray(bias.reshape(KB, P).T))

    in_maps = []
    for c in range(NCORES):
        b, gp = divmod(c, 4)
        qs = slice(gp * QF, (gp + 1) * QF)
        ks = slice(gp * KF, (gp + 1) * KF)
        in_maps.append({
            "xT": xTs[b],
            "xkT": xkTs[b],
            "wqT": np.ascontiguousarray(Wq[qs, :].T.astype(bf)),
            "wkT": np.ascontiguousarray(Wk[ks, :].T.astype(bf)),
            "wvT": np.ascontiguousarray(Wv[ks, :].T.astype(bf)),
            "woT": np.ascontiguousarray(Wo[:, qs].T.astype(bf)),
            "mb": mbs[b],
        })
    return in_maps, KB


def kernel(**inputs):
    from concourse.bass_utils import run_bass_kernel_spmd

    in_maps, KB = _make_in_maps(inputs)
    nc = _get_nc(KB)
    res = run_bass_kernel_spmd(nc, in_maps, core_ids=list(range(NCORES)))
    outs = [np.asarray(r["outT"], dtype=np.float32) for r in res.results]
    out = np.empty((B, S, H), dtype=np.float32)
    for b in range(B):
        acc = outs[4 * b] + outs[4 * b + 1] + outs[4 * b + 2] + outs[4 * b + 3]
        out[b] = acc.T
    return out
